# revision 1
# baseline (speedup 1.0000x reference)
"""Multi-head Koopman module on 8 Trainium2 NeuronCores.

Sharding: core c -> batch b = c//2, head block h0 = 8*(c%2) (8 of 16 heads).
Each (b, h) pair's Gram/cholesky batch lives entirely on one core; the final
W_O contraction is split over head blocks and pair-summed on the host.

Device phase 1: x^T (PE transpose), LN folded into projections (rank-1
-mu*colsum(W) correction inside the PSUM accumulation + rstd scale in the
epilogue), K/V over the prefix, G/M/C_v gram matmuls, Q^T directly.
Host: batched 48x48 cholesky/inverse/svd -> E = B_v L A^2 L^-1 per (k,h).
Device phase 2: acc^T = F Q^T per head, then partial @ W_O.
"""

import math
import os

import numpy as np

B, T, D = 4, 2048, 1024
H, HD = 16, 64
K_OPS, R = 4, 48
POWER_K = 2
LN_EPS = 1e-5
NCORES = 8
HPC = H // 2          # heads per core = 8
NKQ = HPC * R         # 384
NV = HPC * HD         # 512
ND = D // 128         # 8 d-tiles
NT = T // 128         # 16 t-tiles

_f32 = None
_cache = {}


def _mybir():
    from concourse import mybir
    return mybir



def _split_multi_waits(nc):
    """walrus codegen here accepts at most one sync wait per instruction;
    move extra waits onto preceding wait-only NoOps on the same engine."""
    from concourse import mybir
    for fn in nc.m.functions:
        for bb in fn.blocks:
            insts = list(bb.instructions)
            new = []
            changed = False
            for inst in insts:
                si = inst.sync_info
                if si is not None and si.on_wait and len(si.on_wait) > 1:
                    waits = list(si.on_wait)
                    for j, w in enumerate(waits[:-1]):
                        new.append(mybir.InstNoOp(
                            name=f"{inst.name}-ws{j}", engine=inst.engine,
                            ins=[], outs=[],
                            sync_info=mybir.SyncInfo(on_wait=[w], on_update=[])))
                    inst.sync_info = mybir.SyncInfo(on_wait=[waits[-1]],
                                                    on_update=list(si.on_update))
                    changed = True
                new.append(inst)
            if changed:
                bb.instructions = new
    return nc

def _build_phase1(pl: int):
    import concourse.bass as bass
    import concourse.tile as tile
    from concourse import mybir
    from concourse.masks import make_identity
    from contextlib import ExitStack

    f32 = mybir.dt.float32
    f32r = mybir.dt.float32r
    rr = lambda ap: ap.bitcast(f32r)
    nc = bass.Bass()
    xb = nc.dram_tensor("xb", [T, D], f32, kind="ExternalInput")
    wk = nc.dram_tensor("wk", [K_OPS, D, NKQ], f32, kind="ExternalInput")
    wq = nc.dram_tensor("wq", [K_OPS, D, NKQ], f32, kind="ExternalInput")
    wv = nc.dram_tensor("wv", [D, NV], f32, kind="ExternalInput")
    csk = nc.dram_tensor("csk", [K_OPS, 1, NKQ], f32, kind="ExternalInput")
    csq = nc.dram_tensor("csq", [K_OPS, 1, NKQ], f32, kind="ExternalInput")
    csv = nc.dram_tensor("csv", [1, NV], f32, kind="ExternalInput")
    g_out = nc.dram_tensor("g_out", [K_OPS, HPC, R, R], f32, kind="ExternalOutput")
    m_out = nc.dram_tensor("m_out", [K_OPS, HPC, R, R], f32, kind="ExternalOutput")
    cv_out = nc.dram_tensor("cv_out", [K_OPS, HPC, HD, R], f32, kind="ExternalOutput")
    qt_out = nc.dram_tensor("qt_out", [K_OPS, NKQ, T], f32, kind="ExternalOutput")
    rstd_out = nc.dram_tensor("rstd_out", [128, NT], f32, kind="ExternalOutput")

    n_pt = (pl + 127) // 128  # prefix tiles

    with tile.TileContext(nc) as tc, ExitStack() as ctx:
        const = ctx.enter_context(tc.tile_pool(name="const", bufs=1))
        xtp = ctx.enter_context(tc.tile_pool(name="xtp", bufs=1))
        xin = ctx.enter_context(tc.tile_pool(name="xin", bufs=3))
        sqp = ctx.enter_context(tc.tile_pool(name="sqp", bufs=2))
        valsp = ctx.enter_context(tc.tile_pool(name="valsp", bufs=1))
        keysp = ctx.enter_context(tc.tile_pool(name="keysp", bufs=1))
        shiftp = ctx.enter_context(tc.tile_pool(name="shiftp", bufs=1))
        crow = ctx.enter_context(tc.tile_pool(name="crow", bufs=2))
        qstage = ctx.enter_context(tc.tile_pool(name="qstage", bufs=3))

        ident = const.tile([128, 128], f32)
        make_identity(nc, ident)
        ones = const.tile([128, 1], f32)
        nc.vector.memset(ones, 1.0)
        eps_t = const.tile([128, 1], f32)
        nc.vector.memset(eps_t, LN_EPS)

        # persistent x^T tiles: 8 x [128, 2048]
        xT = [xtp.tile([128, T], f32, tag=f"xT{d}", name=f"xT{d}") for d in range(ND)]

        mneg_row = const.tile([1, T], f32)
        s_sp = const.tile([128, NT], f32)
        q_sp = const.tile([128, NT], f32)
        mu_sp = const.tile([128, NT], f32)
        var_sp = const.tile([128, NT], f32)
        rstd_sp = const.tile([128, NT], f32)
        mneg_sp = const.tile([128, NT], f32)

        # ---- stage A: load x, PE-transpose into xT ----
        rows_ctx = tc.tile_pool(name="rowsp", bufs=1)
        rowsp = rows_ctx.__enter__()
        srow = rowsp.tile([1, T], f32)
        qrow = rowsp.tile([1, T], f32)
        with tc.tile_pool(name="tp_ps", bufs=4, space="PSUM") as tp_ps, \
             tc.tile_pool(name="scr_ps", bufs=1, space="PSUM") as scr_ps:
            # dummy matmuls absorb semaphore waits so each transpose
            # (1-wait-limited LDWEIGHTS struct) needs at most one wait
            scr = scr_ps.tile([1, 1], f32)
            nc.tensor.matmul(scr, ident[:, 0:1], ident[:, 0:1], start=True, stop=True)
            for it in range(NT):
                x_tile = xin.tile([128, D], f32)
                nc.sync.dma_start(out=x_tile, in_=xb[it * 128:(it + 1) * 128, :])
                nc.tensor.matmul(scr, x_tile[:, 0:1], x_tile[:, 0:1],
                                 start=True, stop=True)
                for d in range(ND):
                    tp = tp_ps.tile([128, 128], f32)
                    nc.tensor.transpose(tp, x_tile[:, d * 128:(d + 1) * 128], ident)
                    nc.vector.tensor_copy(out=xT[d][:, it * 128:(it + 1) * 128], in_=tp)

        # ---- stage B: LN stats via ones-matmul ----
        with tc.tile_pool(name="st_ps", bufs=2, space="PSUM") as st_ps:
            for c4 in range(4):
                sl = slice(c4 * 512, (c4 + 1) * 512)
                s_ps = st_ps.tile([1, 512], f32, tag="s")
                qq_ps = st_ps.tile([1, 512], f32, tag="q")
                for d in range(ND):
                    nc.tensor.matmul(s_ps, ones, xT[d][:, sl],
                                     start=(d == 0), stop=(d == ND - 1))
                    sq = sqp.tile([128, 512], f32)
                    nc.vector.tensor_mul(sq, xT[d][:, sl], xT[d][:, sl])
                    nc.tensor.matmul(qq_ps, ones, sq,
                                     start=(d == 0), stop=(d == ND - 1))
                nc.vector.tensor_copy(out=srow[0:1, sl], in_=s_ps)
                nc.vector.tensor_copy(out=qrow[0:1, sl], in_=qq_ps)

        for it in range(NT):
            tsl = slice(it * 128, (it + 1) * 128)
            nc.sync.dma_start(out=s_sp[:, it:it + 1], in_=srow[0:1, tsl])
            nc.sync.dma_start(out=q_sp[:, it:it + 1], in_=qrow[0:1, tsl])
        nc.vector.tensor_scalar_mul(mu_sp, s_sp, 1.0 / D)
        nc.vector.tensor_scalar_mul(var_sp, q_sp, 1.0 / D)  # E[x^2]
        nc.vector.tensor_mul(s_sp, mu_sp, mu_sp)            # reuse s_sp = mu^2
        nc.vector.tensor_sub(var_sp, var_sp, s_sp)          # var
        nc.scalar.activation(out=var_sp, in_=var_sp,
                             func=mybir.ActivationFunctionType.Sqrt,
                             bias=eps_t[:, 0:1], scale=1.0)
        nc.vector.reciprocal(rstd_sp, var_sp)
        nc.vector.tensor_scalar_mul(mneg_sp, mu_sp, -1.0)
        for it in range(NT):
            tsl = slice(it * 128, (it + 1) * 128)
            nc.sync.dma_start(out=mneg_row[0:1, tsl], in_=mneg_sp[:, it:it + 1])
        nc.sync.dma_start(out=rstd_out[:, :], in_=rstd_sp)
        rows_ctx.__exit__(None, None, None)

        vals = [valsp.tile([128, NV], f32, tag=f"v{it}", name=f"vals{it}") for it in range(n_pt)]

        # ---- stage C: V projection over prefix ----
        with tc.tile_pool(name="wvp", bufs=1) as wvp, \
             tc.tile_pool(name="vproj_ps", bufs=2, space="PSUM") as vproj_ps:
            wv_sb = wvp.tile([128, ND, NV], f32)
            nc.sync.dma_start(out=wv_sb, in_=wv.rearrange("(a p) n -> p a n", p=128))
            csv_sb = crow.tile([1, NV], f32, tag="csv")
            nc.sync.dma_start(out=csv_sb, in_=csv[:, :])
            for it in range(n_pt):
                tsl = slice(it * 128, (it + 1) * 128)
                vp = vproj_ps.tile([128, NV], f32)
                for d in range(ND):
                    nc.tensor.matmul(vp, xT[d][:, tsl], wv_sb[:, d, :],
                                     start=(d == 0), stop=False)
                nc.tensor.matmul(vp, mneg_row[0:1, tsl], csv_sb, start=False, stop=True)
                nc.scalar.activation(out=vals[it], in_=vp,
                                     func=mybir.ActivationFunctionType.Copy,
                                     bias=0.0, scale=rstd_sp[:, it:it + 1])

        # ---- stage D: per-op K proj, G/M/Cv, Q^T ----
        with tc.tile_pool(name="wkqp", bufs=2) as wkqp, \
             tc.tile_pool(name="proj_ps", bufs=2, space="PSUM") as proj_ps, \
             tc.tile_pool(name="gmcv_ps", bufs=1, space="PSUM") as gmcv_ps, \
             tc.tile_pool(name="qt_ps", bufs=2, space="PSUM") as qt_ps:
            for k in range(K_OPS):
                wk_sb = wkqp.tile([128, ND, NKQ], f32, tag="wk")
                nc.sync.dma_start(out=wk_sb, in_=wk[k].rearrange("(a p) n -> p a n", p=128))
                wq_sb = wkqp.tile([128, ND, NKQ], f32, tag="wq")
                nc.sync.dma_start(out=wq_sb, in_=wq[k].rearrange("(a p) n -> p a n", p=128))
                csk_sb = crow.tile([1, NKQ], f32, tag="csk")
                nc.sync.dma_start(out=csk_sb, in_=csk[k])
                csq_sb = crow.tile([1, NKQ], f32, tag="csq")
                nc.sync.dma_start(out=csq_sb, in_=csq[k])

                keys = [keysp.tile([128, NKQ], f32, tag=f"k{it}", name=f"keys{k}_{it}") for it in range(n_pt)]
                for it in range(n_pt):
                    tsl = slice(it * 128, (it + 1) * 128)
                    kp = proj_ps.tile([128, NKQ], f32)
                    for d in range(ND):
                        nc.tensor.matmul(kp, xT[d][:, tsl], wk_sb[:, d, :],
                                         start=(d == 0), stop=False)
                    nc.tensor.matmul(kp, mneg_row[0:1, tsl], csk_sb,
                                     start=False, stop=True)
                    nc.scalar.activation(out=keys[it], in_=kp,
                                         func=mybir.ActivationFunctionType.Copy,
                                         bias=0.0, scale=rstd_sp[:, it:it + 1])

                # shifted keys for M
                shifts = []
                for it in range(n_pt):
                    rows_m = min(128, max(0, (pl - 1) - 128 * it))
                    if rows_m == 0:
                        shifts.append(None)
                        continue
                    sh = shiftp.tile([128, NKQ], f32, tag=f"s{it}")
                    inner = min(rows_m, 127)
                    nc.sync.dma_start(out=sh[0:inner, :], in_=keys[it][1:1 + inner, :])
                    if rows_m == 128:
                        nc.sync.dma_start(out=sh[127:128, :], in_=keys[it + 1][0:1, :])
                    shifts.append(sh)

                m_its = [it for it in range(n_pt)
                         if min(128, max(0, (pl - 1) - 128 * it)) > 0]
                for h in range(HPC):
                    hs = slice(h * R, (h + 1) * R)
                    vs = slice(h * HD, (h + 1) * HD)
                    gp = gmcv_ps.tile([R, R], f32, tag="g")
                    mp = gmcv_ps.tile([R, R], f32, tag="m")
                    cp = gmcv_ps.tile([HD, R], f32, tag="c")
                    for it in range(n_pt):
                        rows_g = min(128, pl - 128 * it)
                        nc.tensor.matmul(gp, keys[it][0:rows_g, hs], keys[it][0:rows_g, hs],
                                         start=(it == 0), stop=(it == n_pt - 1))
                        nc.tensor.matmul(cp, vals[it][0:rows_g, vs], keys[it][0:rows_g, hs],
                                         start=(it == 0), stop=(it == n_pt - 1))
                        rows_m = min(128, max(0, (pl - 1) - 128 * it))
                        if rows_m > 0:
                            nc.tensor.matmul(mp, shifts[it][0:rows_m, hs],
                                             keys[it][0:rows_m, hs],
                                             start=(it == m_its[0]), stop=(it == m_its[-1]))
                    if not m_its:
                        nc.vector.memset(mp, 0.0)
                    g_sb = qstage.tile([R, R], f32, tag="g_sb", name=f"gsb{k}_{h}")
                    m_sb = qstage.tile([R, R], f32, tag="m_sb", name=f"msb{k}_{h}")
                    c_sb = qstage.tile([HD, R], f32, tag="c_sb", name=f"csb{k}_{h}")
                    nc.vector.tensor_copy(out=g_sb, in_=gp)
                    nc.vector.tensor_copy(out=m_sb, in_=mp)
                    nc.vector.tensor_copy(out=c_sb, in_=cp)
                    nc.sync.dma_start(out=g_out[k, h], in_=g_sb)
                    nc.sync.dma_start(out=m_out[k, h], in_=m_sb)
                    nc.sync.dma_start(out=cv_out[k, h], in_=c_sb)

                # Q^T directly: [128 r-chunk, 512 t-chunk]
                for c3 in range(NKQ // 128):
                    rsl = slice(c3 * 128, (c3 + 1) * 128)
                    for tcn in range(T // 512):
                        tsl = slice(tcn * 512, (tcn + 1) * 512)
                        qp = qt_ps.tile([128, 512], f32)
                        for d in range(ND):
                            nc.tensor.matmul(qp, wq_sb[:, d, rsl], xT[d][:, tsl],
                                             start=(d == 0), stop=False)
                        nc.tensor.matmul(qp, csq_sb[0:1, rsl], mneg_row[0:1, tsl],
                                         start=False, stop=True)
                        qs = qstage.tile([128, 512], f32)
                        nc.vector.tensor_copy(out=qs, in_=qp)
                        nc.sync.dma_start(out=qt_out[k, rsl, tsl], in_=qs)
    return _split_multi_waits(nc)


def _build_phase2():
    import concourse.bass as bass
    import concourse.tile as tile
    from concourse import mybir
    from contextlib import ExitStack

    f32 = mybir.dt.float32
    f32r = mybir.dt.float32r
    rr = lambda ap: ap.bitcast(f32r)
    nc = bass.Bass()
    qt = nc.dram_tensor("qt", [K_OPS, NKQ, T], f32, kind="ExternalInput")
    f_in = nc.dram_tensor("f_in", [R, K_OPS * HPC * HD], f32, kind="ExternalInput")
    wo = nc.dram_tensor("wo", [NV, D], f32, kind="ExternalInput")
    rstd_in = nc.dram_tensor("rstd_in", [128, NT], f32, kind="ExternalInput")
    y_out = nc.dram_tensor("y_out", [T, D], f32, kind="ExternalOutput")

    with tile.TileContext(nc) as tc, ExitStack() as ctx:
        const = ctx.enter_context(tc.tile_pool(name="const", bufs=1))
        qtp = ctx.enter_context(tc.tile_pool(name="qtp", bufs=8))
        accp = ctx.enter_context(tc.tile_pool(name="accp", bufs=2))
        acc_ps = ctx.enter_context(tc.tile_pool(name="acc_ps", bufs=2, space="PSUM"))
        y_ps = ctx.enter_context(tc.tile_pool(name="y_ps", bufs=4, space="PSUM"))

        wo_sb = const.tile([128, 4, D], f32)
        nc.sync.dma_start(out=wo_sb, in_=wo.rearrange("(a p) n -> p a n", p=128))
        f_sb = const.tile([R, K_OPS * HPC * HD], f32)
        nc.sync.dma_start(out=f_sb, in_=f_in[:, :])
        rstd_sb = const.tile([128, NT], f32)
        nc.sync.dma_start(out=rstd_sb, in_=rstd_in[:, :])

        for tcn in range(T // 512):
            tsl = slice(tcn * 512, (tcn + 1) * 512)
            acc_sb = []
            for pair in range(4):
                ap_ps = acc_ps.tile([128, 512], f32)
                for sub in range(2):
                    h = pair * 2 + sub
                    for k in range(K_OPS):
                        qt_sb = qtp.tile([R, 512], f32)
                        nc.sync.dma_start(out=qt_sb, in_=qt[k, h * R:(h + 1) * R, tsl])
                        fsl = slice((k * HPC + h) * HD, (k * HPC + h + 1) * HD)
                        nc.tensor.matmul(ap_ps[sub * 64:(sub + 1) * 64, :],
                                         f_sb[:, fsl], qt_sb,
                                         start=(k == 0), stop=(k == K_OPS - 1))
                a_sb = accp.tile([128, 512], f32, tag=f"a{pair}")
                nc.vector.tensor_copy(out=a_sb, in_=ap_ps)
                acc_sb.append(a_sb)
            for ncn in range(2):
                nsl = slice(ncn * 512, (ncn + 1) * 512)
                for sub4 in range(4):
                    yp = y_ps.tile([128, 512], f32)
                    for pair in range(4):
                        nc.tensor.matmul(yp, acc_sb[pair][:, sub4 * 128:(sub4 + 1) * 128],
                                         wo_sb[:, pair, nsl],
                                         start=(pair == 0), stop=(pair == 3))
                    y_sb = qtp.tile([128, 512], f32, tag="y_sb", name=f"ysb{tcn}_{ncn}_{sub4}")
                    ti = tcn * 4 + sub4
                    nc.scalar.activation(out=y_sb, in_=yp,
                                         func=mybir.ActivationFunctionType.Copy,
                                         bias=0.0, scale=rstd_sb[:, ti:ti + 1])
                    nc.sync.dma_start(
                        out=y_out[tcn * 512 + sub4 * 128: tcn * 512 + (sub4 + 1) * 128, nsl],
                        in_=y_sb)
    return _split_multi_waits(nc)


LAST_PERF = {}


def _numpy_fallback(hidden_states, W_K_ops, W_Q_ops, W_V, W_O, ln_gamma, ln_beta,
                    gate_alphas, gate_alpha, log_ridges, log_gammas, pl):
    x = np.asarray(hidden_states, np.float64)
    mu = x.mean(-1, keepdims=True)
    var = x.var(-1, keepdims=True)
    normed = (x - mu) / np.sqrt(var + LN_EPS) * ln_gamma + ln_beta
    values = (normed @ W_V).reshape(B, T, H, HD).transpose(0, 2, 1, 3)
    acc = np.zeros((B, H, T, HD))
    eye = np.eye(R)
    for k in range(K_OPS):
        ridge = math.exp(float(log_ridges[k]))
        gamma = math.exp(float(log_gammas[k]))
        gate = 1.0 / (1.0 + math.exp(-float(gate_alphas[k])))
        keys = (normed @ W_K_ops[k]).reshape(B, T, H, R).transpose(0, 2, 1, 3)
        qs = (normed @ W_Q_ops[k]).reshape(B, T, H, R).transpose(0, 2, 1, 3)
        pk = keys[:, :, :pl, :]
        G = np.einsum('bhlr,bhls->bhrs', pk, pk) + ridge * eye
        M = np.einsum('bhlr,bhls->bhrs', pk[:, :, 1:, :], pk[:, :, :-1, :])
        L = np.linalg.cholesky(G)
        Linv = np.linalg.inv(L)
        A = Linv @ M @ np.swapaxes(Linv, -1, -2)
        sig = np.linalg.svd(A, compute_uv=False)[..., 0]
        sig = np.maximum(sig, 1e-8)
        scale = min(gamma, 1.0) / np.maximum(sig, 1.0)
        A = A * scale[..., None, None]
        pv = values[:, :, :pl, :]
        Cv = np.einsum('bhld,bhlr->bhdr', pv, pk)
        Ginv = np.swapaxes(Linv, -1, -2) @ Linv
        Bv = Cv @ Ginv
        E = Bv @ L @ A @ A @ Linv
        out_k = np.einsum('bhdr,bhtr->bhtd', E, qs)
        acc = acc + gate * out_k
    out = acc.transpose(0, 2, 1, 3).reshape(B, T, H * HD) @ W_O
    sg = 1.0 / (1.0 + math.exp(-float(np.asarray(gate_alpha).ravel()[0])))
    return (sg * out).astype(np.float32)


def kernel(hidden_states, W_K_ops, W_Q_ops, W_V, W_O, ln_gamma, ln_beta,
           gate_alphas, gate_alpha, log_ridges, log_gammas, prefix_len):
    from concourse.bass_utils import run_bass_kernel_spmd

    hidden_states = np.ascontiguousarray(np.asarray(hidden_states, np.float32))
    W_K_ops = np.asarray(W_K_ops, np.float32)
    W_Q_ops = np.asarray(W_Q_ops, np.float32)
    W_V = np.asarray(W_V, np.float32)
    W_O = np.asarray(W_O, np.float32)
    ln_gamma = np.asarray(ln_gamma, np.float32)
    ln_beta = np.asarray(ln_beta, np.float32)
    gate_alphas = np.asarray(gate_alphas, np.float32)
    log_ridges = np.asarray(log_ridges, np.float32)
    log_gammas = np.asarray(log_gammas, np.float32)
    pl = max(1, min(int(prefix_len), T - 1))

    if np.any(ln_beta != 0) or pl < 2:
        return _numpy_fallback(hidden_states, W_K_ops, W_Q_ops, W_V, W_O,
                               ln_gamma, ln_beta, gate_alphas, gate_alpha,
                               log_ridges, log_gammas, pl)

    # fold LN gamma into the projection weights
    wk_f = W_K_ops * ln_gamma[None, :, None]
    wq_f = W_Q_ops * ln_gamma[None, :, None]
    wv_f = W_V * ln_gamma[:, None]

    in1 = []
    for c in range(NCORES):
        b, h0 = c // 2, (c % 2) * HPC
        wk_c = np.ascontiguousarray(wk_f[:, :, h0 * R:(h0 + HPC) * R])
        wq_c = np.ascontiguousarray(wq_f[:, :, h0 * R:(h0 + HPC) * R])
        wv_c = np.ascontiguousarray(wv_f[:, h0 * HD:(h0 + HPC) * HD])
        in1.append({
            "xb": hidden_states[b],
            "wk": wk_c, "wq": wq_c, "wv": wv_c,
            "csk": np.ascontiguousarray(wk_c.sum(1)[:, None, :]),
            "csq": np.ascontiguousarray(wq_c.sum(1)[:, None, :]),
            "csv": np.ascontiguousarray(wv_c.sum(0)[None, :]),
        })

    key1 = ("p1", pl)
    if key1 not in _cache:
        _cache[key1] = _build_phase1(pl)
    r1 = run_bass_kernel_spmd(_cache[key1], in1, core_ids=list(range(NCORES)))
    LAST_PERF["p1"] = r1

    # ---- host linear algebra on 48x48 blocks ----
    ridge = np.exp(log_ridges.astype(np.float64))
    gamma_k = np.exp(log_gammas.astype(np.float64))
    gates = 1.0 / (1.0 + np.exp(-gate_alphas.astype(np.float64)))
    eye = np.eye(R)
    in2 = []
    for c in range(NCORES):
        b, h0 = c // 2, (c % 2) * HPC
        G = r1.results[c]["g_out"].astype(np.float64) + ridge[:, None, None, None] * eye
        M = r1.results[c]["m_out"].astype(np.float64)
        Cv = r1.results[c]["cv_out"].astype(np.float64)
        L = np.linalg.cholesky(G)
        Linv = np.linalg.inv(L)
        A = Linv @ M @ np.swapaxes(Linv, -1, -2)
        sig = np.linalg.svd(A, compute_uv=False)[..., 0]
        sig = np.maximum(sig, 1e-8)
        scale = np.minimum(gamma_k, 1.0)[:, None] / np.maximum(sig, 1.0)
        A = A * scale[..., None, None]
        Ginv = np.swapaxes(Linv, -1, -2) @ Linv
        Bv = Cv @ Ginv
        E = Bv @ L @ A @ A @ Linv          # [K, HPC, HD, R]
        E = E * gates[:, None, None, None]
        F = np.ascontiguousarray(
            E.transpose(3, 0, 1, 2).reshape(R, K_OPS * HPC * HD).astype(np.float32))
        in2.append({
            "qt": r1.results[c]["qt_out"],
            "f_in": F,
            "wo": np.ascontiguousarray(W_O[h0 * HD:(h0 + HPC) * HD, :]),
            "rstd_in": r1.results[c]["rstd_out"],
        })

    if "p2" not in _cache:
        _cache["p2"] = _build_phase2()
    r2 = run_bass_kernel_spmd(_cache["p2"], in2, core_ids=list(range(NCORES)))
    LAST_PERF["p2"] = r2

    sg = 1.0 / (1.0 + math.exp(-float(np.asarray(gate_alpha).ravel()[0])))
    y = np.empty((B, T, D), np.float32)
    for b in range(B):
        y[b] = (r2.results[2 * b]["y_out"].astype(np.float64)
                + r2.results[2 * b + 1]["y_out"]).astype(np.float32) * np.float32(sg)
    return y



# revision 2
# speedup vs baseline: 1.1709x; 1.1709x over previous
"""Multi-head Koopman module on 8 Trainium2 NeuronCores (v2).

Sharding:
  phase 1: core c -> batch b = c//2, head block h0 = 8*(c%2); computes the
    per-(k,h) Gram G, shifted Gram M and cross-covariance C_v over the prefix.
  host:    batched 48x48 cholesky/inverse/svd -> E = gate * B_v L A^2 L^-1,
    folded into a single output operator W_eff[D, H*HD] per batch.
  phase 2: core c -> batch b = c//2, token half t = c%2; computes
    y = normed @ W_eff @ W_O for its 1024 tokens (full head contraction,
    so outputs concatenate with no cross-core reduction).

Device kernels run LN stats on the Activation engine (accum_out), the
normalize on Vector, PE transposes and all matmuls in float32r (1 cycle/row
at >=256 free dim, 4x the fp32 rate, near-fp32 numerics).  G and M^T share
one matmul (rhs = [keys-window || shifted-window]); C_v is computed
transposed with the same stationary operand.  Chunked results ship to the
host in 6 contiguous DMAs per operator (HWDGE issuance is the scarce
resource: ~625ns per DMA, single shared queue front-end).
"""

import math

import numpy as np

B, T, D = 4, 2048, 1024
H, HD = 16, 64
K_OPS, R = 4, 48
LN_EPS = 1e-5
NCORES = 8
HPC = H // 2          # heads per core in phase 1 = 8
NKQ = HPC * R         # 384
NV = HPC * HD         # 512
ND = D // 128         # 8 d-chunks
TH = T // 2           # tokens per core in phase 2 = 1024
NTH = TH // 128       # 8 token tiles per core in phase 2

# rhs column base per 128-row output chunk for the trimmed 256-wide
# Gram matmuls (chunk c covers key-cols [base, base+256) which contains
# every diagonal block that intersects rows [128c, 128c+128)).
GB = [0, 64, 128]       # G/M chunks over the 384 key dims
CVB = [0, 128, 256]     # C_v^T chunks over the 512 value dims

_cache = {}
LAST_PERF = {}


def _blocks_for_head(h):
    """rows of head h's diagonal block across the 128-row output chunks:
    yields (chunk, row_off_in_chunk, nrows, dst_row0)."""
    r0, r1 = R * h, R * h + R
    out = []
    for c in range(3):
        lo, hi = max(r0, 128 * c), min(r1, 128 * (c + 1))
        if lo < hi:
            out.append((c, lo - 128 * c, hi - lo, lo - r0))
    return out


def _split_multi_waits(nc):
    """walrus codegen accepts at most one sync wait per instruction; move
    extra waits onto preceding wait-only NoOps on the same engine."""
    from concourse import mybir
    for fn in nc.m.functions:
        for bb in fn.blocks:
            insts = list(bb.instructions)
            new = []
            changed = False
            for inst in insts:
                si = inst.sync_info
                if si is not None and si.on_wait and len(si.on_wait) > 1:
                    waits = list(si.on_wait)
                    for j, w in enumerate(waits[:-1]):
                        new.append(mybir.InstNoOp(
                            name=f"{inst.name}-ws{j}", engine=inst.engine,
                            ins=[], outs=[],
                            sync_info=mybir.SyncInfo(on_wait=[w], on_update=[])))
                    inst.sync_info = mybir.SyncInfo(on_wait=[waits[-1]],
                                                    on_update=list(si.on_update))
                    changed = True
                new.append(inst)
            if changed:
                bb.instructions = new
    return nc


def _emit_ln_tile(nc, mybir, lnp, eps_t, x_tile, xn, inv_d):
    """LN stats on ACT (accum_out), normalize on DVE: xn = (x - mu) * rstd."""
    f32 = mybir.dt.float32
    xsq = lnp.tile([128, 2, 512], f32, tag="xsq", name="xsq")
    ssum = lnp.tile([128, 1], f32, tag="ss", name="ssum")
    ssq = lnp.tile([128, 1], f32, tag="sq", name="ssq")
    nc.scalar.activation(out=xsq, in_=x_tile,
                         func=mybir.ActivationFunctionType.Copy,
                         bias=0.0, scale=1.0, accum_out=ssum)
    nc.scalar.activation(out=xsq, in_=x_tile,
                         func=mybir.ActivationFunctionType.Square,
                         bias=0.0, scale=1.0, accum_out=ssq)
    mu = lnp.tile([128, 1], f32, tag="mu", name="mu")
    nc.vector.tensor_scalar_mul(mu, ssum, inv_d)
    musq = lnp.tile([128, 1], f32, tag="m2", name="musq")
    nc.vector.tensor_tensor(out=musq, in0=mu, in1=mu, op=mybir.AluOpType.mult)
    var = lnp.tile([128, 1], f32, tag="va", name="var")
    # var = sumsq/D - mu^2
    nc.vector.scalar_tensor_tensor(
        out=var, in0=ssq, scalar=inv_d, in1=musq,
        op0=mybir.AluOpType.mult, op1=mybir.AluOpType.subtract)
    std = lnp.tile([128, 1], f32, tag="sd", name="std")
    nc.scalar.activation(out=std, in_=var,
                         func=mybir.ActivationFunctionType.Sqrt,
                         bias=eps_t[:, 0:1], scale=1.0)
    rstd = lnp.tile([128, 1], f32, tag="rs", name="rstd")
    nc.vector.reciprocal(rstd, std)
    nc.vector.tensor_scalar(
        out=xn, in0=x_tile, scalar1=mu, scalar2=rstd,
        op0=mybir.AluOpType.subtract, op1=mybir.AluOpType.mult)


def _build_phase1(pt: int):
    """pt = number of 128-row prefix tiles; input xp is [pt*128, D] with
    rows >= prefix_len zeroed on the host."""
    import concourse.bass as bass
    import concourse.tile as tile
    from concourse import mybir
    from concourse.masks import make_identity
    from contextlib import ExitStack

    f32 = mybir.dt.float32
    f32r = mybir.dt.float32r
    PTT = pt * 128

    nc = bass.Bass()
    xp = nc.dram_tensor("xp", [PTT, D], f32, kind="ExternalInput")
    wk = nc.dram_tensor("wk", [K_OPS, D, NKQ], f32r, kind="ExternalInput")
    wv = nc.dram_tensor("wv", [D, NV], f32r, kind="ExternalInput")
    gm_out = nc.dram_tensor("gm_out", [K_OPS, 3, 128, 2, 256], f32,
                            kind="ExternalOutput")
    cvt_out = nc.dram_tensor("cvt_out", [K_OPS, 3, 128, 256], f32,
                             kind="ExternalOutput")

    mm = nc.tensor.matmul

    with tile.TileContext(nc) as tc, ExitStack() as ctx:
        const = ctx.enter_context(tc.tile_pool(name="const", bufs=1))
        xtp = ctx.enter_context(tc.tile_pool(name="xtp", bufs=1))
        xin = ctx.enter_context(tc.tile_pool(name="xin", bufs=4))
        xnp = ctx.enter_context(tc.tile_pool(name="xnp", bufs=3))
        lnp = ctx.enter_context(tc.tile_pool(name="lnp", bufs=3))
        valsp = ctx.enter_context(tc.tile_pool(name="valsp", bufs=1))
        keysp = ctx.enter_context(tc.tile_pool(name="keysp", bufs=2))
        gmsb = ctx.enter_context(tc.tile_pool(name="gmsb", bufs=2))
        wvp = ctx.enter_context(tc.tile_pool(name="wvp", bufs=1))
        wkp = ctx.enter_context(tc.tile_pool(name="wkp", bufs=1))

        ident_f = const.tile([128, 128], f32)
        make_identity(nc, ident_f)
        ident = const.tile([128, 128], f32r)
        nc.vector.tensor_copy(out=ident, in_=ident_f)
        eps_t = const.tile([128, 1], f32)
        nc.vector.memset(eps_t, LN_EPS)
        zrow_f = const.tile([1, NKQ], f32)
        nc.vector.memset(zrow_f, 0.0)
        zrow = const.tile([1, NKQ], f32r)
        nc.vector.tensor_copy(out=zrow, in_=zrow_f)

        # persistent normed-x^T tiles: 8 x [128, PTT]
        xT = [xtp.tile([128, PTT], f32r, tag=f"xT{d}", name=f"xT{d}")
              for d in range(ND)]

        xp_r = xp.rearrange("(t p) (a b) -> t p a b", p=128, a=2)

        # DMA order matters: the x tiles gate the LN pipeline from t=0, the
        # weights are needed only after all transposes -- issue x first
        # (4-deep ring), then wv/wk.
        x_tiles = []
        for it in range(pt):
            x_tile = xin.tile([128, 2, 512], f32, tag="x", name=f"xt{it}")
            nc.sync.dma_start(out=x_tile, in_=xp_r[it])
            x_tiles.append(x_tile)
        wv_sb = wvp.tile([128, ND, NV], f32r)
        nc.sync.dma_start(out=wv_sb, in_=wv.rearrange("(a p) n -> p a n", p=128))
        wk_sbs = {}

        def load_wk(k):
            t = wkp.tile([128, ND, NKQ], f32r, tag=f"wk{k % 2}", name=f"wk{k}")
            nc.sync.dma_start(out=t,
                              in_=wk[k].rearrange("(a p) n -> p a n", p=128))
            wk_sbs[k] = t

        load_wk(0)
        load_wk(1)

        # ---- LN + transpose per prefix tile ----
        with tc.tile_pool(name="tp_ps", bufs=4, space="PSUM") as tp_ps, \
             tc.tile_pool(name="scr_ps", bufs=1, space="PSUM") as scr_ps:
            scr = scr_ps.tile([1, 1], f32)
            nc.tensor.matmul(scr, ident_f[:, 0:1], ident_f[:, 0:1],
                             start=True, stop=True)
            for it in range(pt):
                xn = xnp.tile([128, 2, 512], f32r, tag="xn")
                _emit_ln_tile(nc, mybir, lnp, eps_t, x_tiles[it], xn, 1.0 / D)
                # dummy matmul absorbs semaphore waits so each transpose
                # (1-wait-limited LDWEIGHTS struct) needs at most one wait
                nc.tensor.matmul(scr, ident_f[:, 0:1], ident_f[:, 0:1],
                                 start=True, stop=True)
                for d in range(ND):
                    tp = tp_ps.tile([128, 128], f32r)
                    sl = xn[:, d // 4, (d % 4) * 128:(d % 4) * 128 + 128]
                    nc.tensor.transpose(tp, sl, ident)
                    nc.vector.tensor_copy(out=xT[d][:, it * 128:(it + 1) * 128],
                                          in_=tp)

        # ---- V projection over prefix ----
        vals = [valsp.tile([128, NV], f32r, tag=f"v{it}", name=f"vals{it}")
                for it in range(pt)]
        with tc.tile_pool(name="vproj_ps", bufs=2, space="PSUM") as vproj_ps:
            for it in range(pt):
                tsl = slice(it * 128, (it + 1) * 128)
                vp = vproj_ps.tile([128, NV], f32)
                for d in range(ND):
                    mm(vp, xT[d][:, tsl], wv_sb[:, d, :],
                       start=(d == 0), stop=(d == ND - 1))
                nc.scalar.copy(out=vals[it], in_=vp)

        # ---- per-op: K proj, shift, [G | M^T] and C_v^T ----
        with tc.tile_pool(name="kproj_ps", bufs=2, space="PSUM") as kproj_ps, \
             tc.tile_pool(name="gm_ps", bufs=1, space="PSUM") as gm_ps:
            for k in range(K_OPS):
                wk_sb = wk_sbs[k]
                # ks[:, 0, it, :] = keys tile it; ks[:, 1, it, :] = shifted keys
                ks = keysp.tile([128, 2, pt, NKQ], f32r, tag="ks", name=f"ks{k}")
                for it in range(pt):
                    tsl = slice(it * 128, (it + 1) * 128)
                    kp = kproj_ps.tile([128, NKQ], f32)
                    for d in range(ND):
                        mm(kp, xT[d][:, tsl], wk_sb[:, d, :],
                           start=(d == 0), stop=(d == ND - 1))
                    nc.scalar.copy(out=ks[:, 0, it, :], in_=kp)
                if k + 2 < K_OPS:
                    load_wk(k + 2)

                # shifted keys in 3 DMAs (row l holds key l+1; zero padding
                # beyond the prefix makes the tail terms vanish automatically)
                nc.sync.dma_start(out=ks[0:127, 1, :, :], in_=ks[1:128, 0, :, :])
                if pt > 1:
                    nc.sync.dma_start(out=ks[127:128, 1, 0:pt - 1, :],
                                      in_=ks[0:1, 0, 1:pt, :])
                nc.sync.dma_start(out=ks[127:128, 1, pt - 1, :], in_=zrow)

                # [G | M^T] per chunk: lhsT = keys chunk, rhs = (keys, shifted)
                # 256-wide windows; C_v^T: same lhsT, rhs = vals window.
                gm = [gm_ps.tile([128, 2, 256], f32, tag=f"gm{c}",
                                 name=f"gm{k}_{c}") for c in range(3)]
                cvt = [gm_ps.tile([128, 256], f32, tag=f"cv{c}",
                                  name=f"cvt{k}_{c}") for c in range(3)]
                for it in range(pt):
                    for c in range(3):
                        lhs = ks[:, 0, it, c * 128:(c + 1) * 128]
                        mm(gm[c], lhs, ks[:, :, it, GB[c]:GB[c] + 256],
                           start=(it == 0), stop=(it == pt - 1))
                        mm(cvt[c], lhs, vals[it][:, CVB[c]:CVB[c] + 256],
                           start=(it == 0), stop=(it == pt - 1))
                for c in range(3):
                    gm_sb = gmsb.tile([128, 2, 256], f32, tag=f"gs{c}",
                                      name=f"gmsb{k}_{c}")
                    nc.vector.tensor_copy(out=gm_sb, in_=gm[c])
                    nc.sync.dma_start(out=gm_out[k, c], in_=gm_sb)
                    cv_sb = gmsb.tile([128, 256], f32, tag=f"cs{c}",
                                      name=f"cvsb{k}_{c}")
                    nc.vector.tensor_copy(out=cv_sb, in_=cvt[c])
                    nc.sync.dma_start(out=cvt_out[k, c], in_=cv_sb)
    return _split_multi_waits(nc)


def _build_phase2():
    import concourse.bass as bass
    import concourse.tile as tile
    from concourse import mybir
    from concourse.masks import make_identity
    from contextlib import ExitStack

    f32 = mybir.dt.float32
    f32r = mybir.dt.float32r

    nc = bass.Bass()
    xh = nc.dram_tensor("xh", [TH, D], f32, kind="ExternalInput")
    weff = nc.dram_tensor("weff", [D, H * HD], f32r, kind="ExternalInput")
    wo = nc.dram_tensor("wo", [H * HD, D], f32r, kind="ExternalInput")
    y_out = nc.dram_tensor("y_out", [TH, D], f32, kind="ExternalOutput")

    mm = nc.tensor.matmul
    NHD = (H * HD) // 128  # 8 head-dim chunks

    with tile.TileContext(nc) as tc, ExitStack() as ctx:
        const = ctx.enter_context(tc.tile_pool(name="const", bufs=1))
        wp = ctx.enter_context(tc.tile_pool(name="wp", bufs=1))
        xtp = ctx.enter_context(tc.tile_pool(name="xtp", bufs=1))
        xin = ctx.enter_context(tc.tile_pool(name="xin", bufs=1))
        lnp = ctx.enter_context(tc.tile_pool(name="lnp", bufs=3))
        xnp = ctx.enter_context(tc.tile_pool(name="xnp", bufs=3))
        ysb = ctx.enter_context(tc.tile_pool(name="ysb", bufs=3))

        ident_f = const.tile([128, 128], f32)
        make_identity(nc, ident_f)
        ident = const.tile([128, 128], f32r)
        nc.vector.tensor_copy(out=ident, in_=ident_f)
        eps_t = const.tile([128, 1], f32)
        nc.vector.memset(eps_t, LN_EPS)

        xT = [xtp.tile([128, TH], f32r, tag=f"xT{d}", name=f"xT{d}")
              for d in range(ND)]
        accT = [xtp.tile([128, TH], f32r, tag=f"aT{j}", name=f"accT{j}")
                for j in range(NHD)]
        weff_sb = [wp.tile([128, H * HD], f32r, tag=f"we{i}", name=f"we{i}")
                   for i in range(ND)]
        wo_sb = wp.tile([128, NHD, D], f32r)

        xh_r = xh.rearrange("(t p) (a b) -> t p a b", p=128, a=2)

        # DMA order: x tiles gate LN from t=0; weff chunks gate accT (~15us
        # in); wo gates the y stage (~33us in)
        x_tiles = []
        for it in range(NTH):
            x_tile = xin.tile([128, 2, 512], f32, tag=f"x{it}", name=f"xt{it}")
            nc.sync.dma_start(out=x_tile, in_=xh_r[it])
            x_tiles.append(x_tile)
        for i in range(ND):
            nc.sync.dma_start(out=weff_sb[i],
                              in_=weff[i * 128:(i + 1) * 128, :])
        wo_r = wo.rearrange("(a p) n -> p a n", p=128)
        nc.sync.dma_start(out=wo_sb[:, 0:4, :], in_=wo_r[:, 0:4, :])
        nc.sync.dma_start(out=wo_sb[:, 4:8, :], in_=wo_r[:, 4:8, :])

        with tc.tile_pool(name="tp_ps", bufs=4, space="PSUM") as tp_ps, \
             tc.tile_pool(name="scr_ps", bufs=1, space="PSUM") as scr_ps:
            scr = scr_ps.tile([1, 1], f32)
            nc.tensor.matmul(scr, ident_f[:, 0:1], ident_f[:, 0:1],
                             start=True, stop=True)
            for it in range(NTH):
                xn = xnp.tile([128, 2, 512], f32r, tag="xn")
                _emit_ln_tile(nc, mybir, lnp, eps_t, x_tiles[it], xn, 1.0 / D)
                nc.tensor.matmul(scr, ident_f[:, 0:1], ident_f[:, 0:1],
                                 start=True, stop=True)
                for d in range(ND):
                    tp = tp_ps.tile([128, 128], f32r)
                    sl = xn[:, d // 4, (d % 4) * 128:(d % 4) * 128 + 128]
                    nc.tensor.transpose(tp, sl, ident)
                    nc.vector.tensor_copy(out=xT[d][:, it * 128:(it + 1) * 128],
                                          in_=tp)

        # acc^T[j] = sum_i W_eff[i-chunk, j-slice]^T @ normed^T[i-chunk, :]
        # i-outer so the accumulation streams with the weff chunk arrivals
        # (4 PSUM banks per half-pass); then
        # y[tile] = sum_j acc^T[j, tile-slice]^T @ W_O[j-chunk, :]
        with tc.tile_pool(name="acc_ps", bufs=1, space="PSUM") as acc_ps, \
             tc.tile_pool(name="y_ps", bufs=3, space="PSUM") as y_ps:
            for tch in range(TH // 512):
                tsl = slice(tch * 512, (tch + 1) * 512)
                for jh in range(2):
                    pss = [acc_ps.tile([128, 512], f32, tag=f"a{jj}",
                                       name=f"acc{tch}_{jh}_{jj}")
                           for jj in range(4)]
                    for i in range(ND):
                        for jj in range(4):
                            j = jh * 4 + jj
                            mm(pss[jj], weff_sb[i][:, j * 128:(j + 1) * 128],
                               xT[i][:, tsl],
                               start=(i == 0), stop=(i == ND - 1))
                    for jj in range(4):
                        nc.vector.tensor_copy(
                            out=accT[jh * 4 + jj][:, tsl], in_=pss[jj])
                for it in range(tch * 4, tch * 4 + 4):
                    ysl = slice(it * 128, (it + 1) * 128)
                    y_sb = ysb.tile([128, 2, 512], f32)
                    for ch in range(D // 512):
                        csl = slice(ch * 512, (ch + 1) * 512)
                        ps = y_ps.tile([128, 512], f32)
                        for j in range(NHD):
                            mm(ps, accT[j][:, ysl], wo_sb[:, j, csl],
                               start=(j == 0), stop=(j == NHD - 1))
                        nc.scalar.copy(out=y_sb[:, ch, :], in_=ps)
                    nc.sync.dma_start(
                        out=y_out[ysl, :].rearrange("p (a b) -> p a b", a=2),
                        in_=y_sb)
    return _split_multi_waits(nc)


def _numpy_fallback(hidden_states, W_K_ops, W_Q_ops, W_V, W_O, ln_gamma, ln_beta,
                    gate_alphas, gate_alpha, log_ridges, log_gammas, pl):
    x = np.asarray(hidden_states, np.float64)
    mu = x.mean(-1, keepdims=True)
    var = x.var(-1, keepdims=True)
    normed = (x - mu) / np.sqrt(var + LN_EPS) * ln_gamma + ln_beta
    values = (normed @ W_V).reshape(B, T, H, HD).transpose(0, 2, 1, 3)
    acc = np.zeros((B, H, T, HD))
    eye = np.eye(R)
    for k in range(K_OPS):
        ridge = math.exp(float(log_ridges[k]))
        gamma = math.exp(float(log_gammas[k]))
        gate = 1.0 / (1.0 + math.exp(-float(gate_alphas[k])))
        keys = (normed @ W_K_ops[k]).reshape(B, T, H, R).transpose(0, 2, 1, 3)
        qs = (normed @ W_Q_ops[k]).reshape(B, T, H, R).transpose(0, 2, 1, 3)
        pk = keys[:, :, :pl, :]
        G = np.einsum('bhlr,bhls->bhrs', pk, pk) + ridge * eye
        M = np.einsum('bhlr,bhls->bhrs', pk[:, :, 1:, :], pk[:, :, :-1, :])
        L = np.linalg.cholesky(G)
        Linv = np.linalg.inv(L)
        A = Linv @ M @ np.swapaxes(Linv, -1, -2)
        sig = np.linalg.svd(A, compute_uv=False)[..., 0]
        sig = np.maximum(sig, 1e-8)
        scale = min(gamma, 1.0) / np.maximum(sig, 1.0)
        A = A * scale[..., None, None]
        pv = values[:, :, :pl, :]
        Cv = np.einsum('bhld,bhlr->bhdr', pv, pk)
        Ginv = np.swapaxes(Linv, -1, -2) @ Linv
        Bv = Cv @ Ginv
        E = Bv @ L @ A @ A @ Linv
        out_k = np.einsum('bhdr,bhtr->bhtd', E, qs)
        acc = acc + gate * out_k
    out = acc.transpose(0, 2, 1, 3).reshape(B, T, H * HD) @ W_O
    sg = 1.0 / (1.0 + math.exp(-float(np.asarray(gate_alpha).ravel()[0])))
    return (sg * out).astype(np.float32)


def _extract_gmcv(res):
    """host-side: pull the per-head diagonal blocks out of the chunked
    [G | M^T] and C_v^T device outputs."""
    gm = res["gm_out"]      # [K, 3, 128, 2, 256]
    cvt = res["cvt_out"]    # [K, 3, 128, 256]
    G = np.empty((K_OPS, HPC, R, R), np.float64)
    M = np.empty((K_OPS, HPC, R, R), np.float64)
    Cv = np.empty((K_OPS, HPC, HD, R), np.float64)
    for h in range(HPC):
        for (c, off, nr, dst) in _blocks_for_head(h):
            gcs = slice(R * h - GB[c], R * h - GB[c] + R)
            G[:, h, dst:dst + nr, :] = gm[:, c, off:off + nr, 0, gcs]
            # device computed M^T = keys^T @ shifted; transpose back
            M[:, h, :, dst:dst + nr] = np.swapaxes(
                gm[:, c, off:off + nr, 1, gcs], -1, -2)
            vcs = slice(HD * h - CVB[c], HD * h - CVB[c] + HD)
            Cv[:, h, :, dst:dst + nr] = np.swapaxes(
                cvt[:, c, off:off + nr, vcs], -1, -2)
    return G, M, Cv


def kernel(hidden_states, W_K_ops, W_Q_ops, W_V, W_O, ln_gamma, ln_beta,
           gate_alphas, gate_alpha, log_ridges, log_gammas, prefix_len):
    from concourse.bass_utils import run_bass_kernel_spmd

    hidden_states = np.ascontiguousarray(np.asarray(hidden_states, np.float32))
    W_K_ops = np.asarray(W_K_ops, np.float32)
    W_Q_ops = np.asarray(W_Q_ops, np.float32)
    W_V = np.asarray(W_V, np.float32)
    W_O = np.ascontiguousarray(np.asarray(W_O, np.float32))
    ln_gamma = np.asarray(ln_gamma, np.float32)
    ln_beta = np.asarray(ln_beta, np.float32)
    gate_alphas = np.asarray(gate_alphas, np.float32)
    log_ridges = np.asarray(log_ridges, np.float32)
    log_gammas = np.asarray(log_gammas, np.float32)
    pl = max(1, min(int(prefix_len), T - 1))
    pt = (pl + 127) // 128

    if np.any(ln_beta != 0) or pl < 2:
        return _numpy_fallback(hidden_states, W_K_ops, W_Q_ops, W_V, W_O,
                               ln_gamma, ln_beta, gate_alphas, gate_alpha,
                               log_ridges, log_gammas, pl)

    # fold LN gamma into the projection weights
    wk_f = W_K_ops * ln_gamma[None, :, None]
    wq_f = W_Q_ops * ln_gamma[None, :, None]
    wv_f = W_V * ln_gamma[:, None]

    if pl == pt * 128:
        xpad = hidden_states[:, :pl]
    else:
        xpad = np.zeros((B, pt * 128, D), np.float32)
        xpad[:, :pl] = hidden_states[:, :pl]

    in1 = []
    for c in range(NCORES):
        b, h0 = c // 2, (c % 2) * HPC
        in1.append({
            "xp": xpad[b],
            "wk": np.ascontiguousarray(wk_f[:, :, h0 * R:(h0 + HPC) * R]),
            "wv": np.ascontiguousarray(wv_f[:, h0 * HD:(h0 + HPC) * HD]),
        })

    key1 = ("p1", pt)
    if key1 not in _cache:
        _cache[key1] = _build_phase1(pt)
    r1 = run_bass_kernel_spmd(_cache[key1], in1, core_ids=list(range(NCORES)))
    LAST_PERF["p1"] = r1

    # ---- host linear algebra on 48x48 blocks -> W_eff per batch ----
    ridge = np.exp(log_ridges.astype(np.float64))
    gamma_k = np.exp(log_gammas.astype(np.float64))
    gates = 1.0 / (1.0 + np.exp(-gate_alphas.astype(np.float64)))
    sg = 1.0 / (1.0 + math.exp(-float(np.asarray(gate_alpha).ravel()[0])))
    eye = np.eye(R)

    E_full = np.empty((B, K_OPS, H, HD, R))
    for c in range(NCORES):
        b, h0 = c // 2, (c % 2) * HPC
        G, M, Cv = _extract_gmcv(r1.results[c])
        G = G + ridge[:, None, None, None] * eye
        L = np.linalg.cholesky(G)
        Linv = np.linalg.inv(L)
        A = Linv @ M @ np.swapaxes(Linv, -1, -2)
        sig = np.linalg.svd(A, compute_uv=False)[..., 0]
        sig = np.maximum(sig, 1e-8)
        scale = np.minimum(gamma_k, 1.0)[:, None] / np.maximum(sig, 1.0)
        A = A * scale[..., None, None]
        Ginv = np.swapaxes(Linv, -1, -2) @ Linv
        Bv = Cv @ Ginv
        E = Bv @ L @ A @ A @ Linv          # [K, HPC, HD, R]
        E = E * (sg * gates)[:, None, None, None]
        E_full[b, :, h0:h0 + HPC] = E

    # W_eff[b] = sum_k Wq_f[k][:, h-block] @ E[b, k, h]^T   -> [D, H*HD]
    wq_h = wq_f.reshape(K_OPS, D, H, R).transpose(0, 2, 1, 3)  # [K, H, D, R]
    weffs = []
    for b in range(B):
        w = np.zeros((H, D, HD), np.float64)
        for k in range(K_OPS):
            w += wq_h[k].astype(np.float64) @ E_full[b, k].transpose(0, 2, 1)
        weffs.append(np.ascontiguousarray(
            w.transpose(1, 0, 2).reshape(D, H * HD).astype(np.float32)))

    in2 = []
    for c in range(NCORES):
        b, th = c // 2, c % 2
        in2.append({
            "xh": hidden_states[b, th * TH:(th + 1) * TH],
            "weff": weffs[b],
            "wo": W_O,
        })

    if "p2" not in _cache:
        _cache["p2"] = _build_phase2()
    r2 = run_bass_kernel_spmd(_cache["p2"], in2, core_ids=list(range(NCORES)))
    LAST_PERF["p2"] = r2

    y = np.empty((B, T, D), np.float32)
    for c in range(NCORES):
        b, th = c // 2, c % 2
        y[b, th * TH:(th + 1) * TH] = r2.results[c]["y_out"]
    return y


# revision 3
# speedup vs baseline: 1.2473x; 1.0652x over previous
"""Multi-head Koopman module on 8 Trainium2 NeuronCores (v3).

Sharding:
  phase 1: core c -> batch b = c//2, head block h0 = 8*(c%2); computes the
    per-(k,h) Gram G, shifted Gram M and cross-covariance C_v over the prefix.
  host:    batched 48x48 cholesky/inverse/svd -> E = gate * B_v L A^2 L^-1,
    folded into a single output operator W_eff[D, H*HD] per batch.
  phase 2: core c -> batch b = c//2, token half t = c%2; computes
    y = normed @ W_eff @ W_O for its 1024 tokens (full head contraction,
    so outputs concatenate with no cross-core reduction).

v3 data path: bf16 activations/weights (fp32 PSUM accumulation), which
halves the DMA traffic and keeps every matmul at 1 cycle/row.  LayerNorm
never blocks the PE: raw x tiles are transposed as soon as they land, the
mean is folded in as a rank-1 PSUM correction (-mu x colsum(W)) and the
1/std as the per-token scale of the PSUM->SBUF copy.  G and M^T share one
matmul (rhs = [keys-window || shifted-window]); C_v is computed transposed
with the same stationary operand; chunked f32 results ship to the host in
6 contiguous DMAs per operator.
"""

import math

import numpy as np

B, T, D = 4, 2048, 1024
H, HD = 16, 64
K_OPS, R = 4, 48
LN_EPS = 1e-5
NCORES = 8
HPC = H // 2          # heads per core in phase 1 = 8
NKQ = HPC * R         # 384
NV = HPC * HD         # 512
ND = D // 128         # 8 d-chunks
TH = T // 2           # tokens per core in phase 2 = 1024
NTH = TH // 128       # 8 token tiles per core in phase 2

GB = [0, 64, 128]       # G/M chunk rhs col bases over the 384 key dims
CVB = [0, 128, 256]     # C_v^T chunk rhs col bases over the 512 value dims

_cache = {}
LAST_PERF = {}


def _bf16(a):
    import ml_dtypes
    return np.asarray(a, np.float32).astype(ml_dtypes.bfloat16)


def _blocks_for_head(h):
    r0, r1 = R * h, R * h + R
    out = []
    for c in range(3):
        lo, hi = max(r0, 128 * c), min(r1, 128 * (c + 1))
        if lo < hi:
            out.append((c, lo - 128 * c, hi - lo, lo - r0))
    return out


def _split_multi_waits(nc):
    """walrus codegen accepts at most one sync wait per instruction; move
    extra waits onto preceding wait-only NoOps on the same engine."""
    from concourse import mybir
    for fn in nc.m.functions:
        for bb in fn.blocks:
            insts = list(bb.instructions)
            new = []
            changed = False
            for inst in insts:
                si = inst.sync_info
                if si is not None and si.on_wait and len(si.on_wait) > 1:
                    waits = list(si.on_wait)
                    for j, w in enumerate(waits[:-1]):
                        new.append(mybir.InstNoOp(
                            name=f"{inst.name}-ws{j}", engine=inst.engine,
                            ins=[], outs=[],
                            sync_info=mybir.SyncInfo(on_wait=[w], on_update=[])))
                    inst.sync_info = mybir.SyncInfo(on_wait=[waits[-1]],
                                                    on_update=list(si.on_update))
                    changed = True
                new.append(inst)
            if changed:
                bb.instructions = new
    return nc


def _emit_ln_stats(nc, mybir, lnp, eps_t, x_tile, it, negmu_row, rstds, inv_d):
    """LN stats only (PE-independent): ACT accum passes -> negmu/rstd.
    negmu lands (bf16) in negmu_row[0, it*128:(it+1)*128] via a transposing
    DMA; rstd stays as a per-tile [128, 1] f32 column (ACT copy scale)."""
    f32 = mybir.dt.float32
    bf16 = mybir.dt.bfloat16
    xsq = lnp.tile([128, 2, 512], f32, tag="xsq", name="xsq")
    ssum = lnp.tile([128, 1], f32, tag="ss", name="ssum")
    ssq = lnp.tile([128, 1], f32, tag="sq", name="ssq")
    nc.scalar.activation(out=xsq, in_=x_tile,
                         func=mybir.ActivationFunctionType.Copy,
                         bias=0.0, scale=1.0, accum_out=ssum)
    nc.scalar.activation(out=xsq, in_=x_tile,
                         func=mybir.ActivationFunctionType.Square,
                         bias=0.0, scale=1.0, accum_out=ssq)
    negmu = lnp.tile([128, 1], f32, tag="nm", name="negmu")
    nc.vector.tensor_scalar_mul(negmu, ssum, -inv_d)
    negmu_b = lnp.tile([128, 1], bf16, tag="nb", name="negmu_b")
    nc.vector.tensor_copy(out=negmu_b, in_=negmu)
    nc.sync.dma_start(out=negmu_row[0:1, it * 128:(it + 1) * 128],
                      in_=negmu_b)
    musq = lnp.tile([128, 1], f32, tag="m2", name="musq")
    nc.vector.tensor_tensor(out=musq, in0=negmu, in1=negmu,
                            op=mybir.AluOpType.mult)
    var = lnp.tile([128, 1], f32, tag="va", name="var")
    nc.vector.scalar_tensor_tensor(
        out=var, in0=ssq, scalar=inv_d, in1=musq,
        op0=mybir.AluOpType.mult, op1=mybir.AluOpType.subtract)
    std = lnp.tile([128, 1], f32, tag="sd", name="std")
    nc.scalar.activation(out=std, in_=var,
                         func=mybir.ActivationFunctionType.Sqrt,
                         bias=eps_t[:, 0:1], scale=1.0)
    rstd = rstds[it]
    nc.vector.reciprocal(rstd, std)


def _build_phase1(pt: int):
    """pt = number of 128-row prefix tiles; input xp is [pt*128, D] bf16
    with rows >= prefix_len zeroed on the host."""
    import concourse.bass as bass
    import concourse.tile as tile
    from concourse import mybir
    from concourse.masks import make_identity
    from contextlib import ExitStack

    f32 = mybir.dt.float32
    bf16 = mybir.dt.bfloat16
    PTT = pt * 128

    nc = bass.Bass()
    xp = nc.dram_tensor("xp", [PTT, D], bf16, kind="ExternalInput")
    wk = nc.dram_tensor("wk", [K_OPS, D, NKQ], bf16, kind="ExternalInput")
    wv = nc.dram_tensor("wv", [D, NV], bf16, kind="ExternalInput")
    csk = nc.dram_tensor("csk", [K_OPS, 1, NKQ], bf16, kind="ExternalInput")
    csv = nc.dram_tensor("csv", [1, NV], bf16, kind="ExternalInput")
    gm_out = nc.dram_tensor("gm_out", [K_OPS, 3, 128, 2, 256], f32,
                            kind="ExternalOutput")
    cvt_out = nc.dram_tensor("cvt_out", [K_OPS, 3, 128, 256], f32,
                             kind="ExternalOutput")

    mm = nc.tensor.matmul

    with tile.TileContext(nc) as tc, ExitStack() as ctx:
        const = ctx.enter_context(tc.tile_pool(name="const", bufs=1))
        xtp = ctx.enter_context(tc.tile_pool(name="xtp", bufs=1))
        xin = ctx.enter_context(tc.tile_pool(name="xin", bufs=4))
        lnp = ctx.enter_context(tc.tile_pool(name="lnp", bufs=3))
        rsp = ctx.enter_context(tc.tile_pool(name="rsp", bufs=1))
        valsp = ctx.enter_context(tc.tile_pool(name="valsp", bufs=1))
        keysp = ctx.enter_context(tc.tile_pool(name="keysp", bufs=2))
        gmsb = ctx.enter_context(tc.tile_pool(name="gmsb", bufs=2))
        wvp = ctx.enter_context(tc.tile_pool(name="wvp", bufs=1))
        wkp = ctx.enter_context(tc.tile_pool(name="wkp", bufs=1))

        ident_f = const.tile([128, 128], f32)
        make_identity(nc, ident_f)
        ident = const.tile([128, 128], bf16)
        nc.vector.tensor_copy(out=ident, in_=ident_f)
        eps_t = const.tile([128, 1], f32)
        nc.vector.memset(eps_t, LN_EPS)
        zrow_f = const.tile([1, NKQ], f32)
        nc.vector.memset(zrow_f, 0.0)
        zrow = const.tile([1, NKQ], bf16)
        nc.vector.tensor_copy(out=zrow, in_=zrow_f)
        negmu_row = const.tile([1, PTT], bf16)
        rstds = [rsp.tile([128, 1], f32, tag=f"r{it}", name=f"rstd{it}")
                 for it in range(pt)]

        # raw-x^T tiles (normalization folded into the projections)
        xT = [xtp.tile([128, PTT], bf16, tag=f"xT{d}", name=f"xT{d}")
              for d in range(ND)]

        xp_r = xp.rearrange("(t p) (a b) -> t p a b", p=128, a=2)

        # DMA order: x tiles stream first (gate transposes from t=0), then
        # the projection weights (needed only after the transposes)
        x_tiles = []
        for it in range(pt):
            x_tile = xin.tile([128, 2, 512], bf16, tag="x", name=f"xt{it}")
            nc.sync.dma_start(out=x_tile, in_=xp_r[it])
            x_tiles.append(x_tile)
        wv_sb = wvp.tile([128, ND, NV], bf16)
        nc.sync.dma_start(out=wv_sb, in_=wv.rearrange("(a p) n -> p a n", p=128))
        csv_sb = const.tile([1, NV], bf16)
        nc.sync.dma_start(out=csv_sb, in_=csv[:, :])
        csk_sb = const.tile([1, K_OPS, NKQ], bf16)
        nc.sync.dma_start(out=csk_sb, in_=csk.rearrange("k o n -> o k n"))
        wk_sbs = {}

        def load_wk(k):
            t = wkp.tile([128, ND, NKQ], bf16, tag=f"wk{k % 2}", name=f"wk{k}")
            nc.sync.dma_start(out=t,
                              in_=wk[k].rearrange("(a p) n -> p a n", p=128))
            wk_sbs[k] = t

        load_wk(0)
        load_wk(1)

        # ---- transpose raw x as tiles land; LN stats run beside on ACT/DVE
        with tc.tile_pool(name="tp_ps", bufs=4, space="PSUM") as tp_ps, \
             tc.tile_pool(name="scr_ps", bufs=1, space="PSUM") as scr_ps:
            scr = scr_ps.tile([1, 1], f32)
            nc.tensor.matmul(scr, ident_f[:, 0:1], ident_f[:, 0:1],
                             start=True, stop=True)
            for it in range(pt):
                x_tile = x_tiles[it]
                _emit_ln_stats(nc, mybir, lnp, eps_t, x_tile, it,
                               negmu_row, rstds, 1.0 / D)
                # dummy matmul absorbs semaphore waits so each transpose
                # (1-wait-limited LDWEIGHTS struct) needs at most one wait
                nc.tensor.matmul(scr, ident_f[:, 0:1], ident_f[:, 0:1],
                                 start=True, stop=True)
                for d in range(ND):
                    tp = tp_ps.tile([128, 128], bf16)
                    sl = x_tile[:, d // 4, (d % 4) * 128:(d % 4) * 128 + 128]
                    nc.tensor.transpose(tp, sl, ident)
                    nc.vector.tensor_copy(out=xT[d][:, it * 128:(it + 1) * 128],
                                          in_=tp)

        # ---- V projection over prefix (rank-1 mean fold + rstd scale) ----
        vals = [valsp.tile([128, NV], bf16, tag=f"v{it}", name=f"vals{it}")
                for it in range(pt)]
        with tc.tile_pool(name="vproj_ps", bufs=2, space="PSUM") as vproj_ps:
            for it in range(pt):
                tsl = slice(it * 128, (it + 1) * 128)
                vp = vproj_ps.tile([128, NV], f32)
                for d in range(ND):
                    mm(vp, xT[d][:, tsl], wv_sb[:, d, :],
                       start=(d == 0), stop=False)
                mm(vp, negmu_row[0:1, tsl], csv_sb, start=False, stop=True)
                nc.scalar.activation(out=vals[it], in_=vp,
                                     func=mybir.ActivationFunctionType.Copy,
                                     bias=0.0, scale=rstds[it][:, 0:1])

        # ---- per-op: K proj, shift, [G | M^T] and C_v^T ----
        with tc.tile_pool(name="kproj_ps", bufs=2, space="PSUM") as kproj_ps, \
             tc.tile_pool(name="gm_ps", bufs=1, space="PSUM") as gm_ps:
            for k in range(K_OPS):
                wk_sb = wk_sbs[k]
                # ks[:, 0, it, :] = keys tile it; ks[:, 1, it, :] = shifted
                ks = keysp.tile([128, 2, pt, NKQ], bf16, tag="ks", name=f"ks{k}")
                for it in range(pt):
                    tsl = slice(it * 128, (it + 1) * 128)
                    kp = kproj_ps.tile([128, NKQ], f32)
                    for d in range(ND):
                        mm(kp, xT[d][:, tsl], wk_sb[:, d, :],
                           start=(d == 0), stop=False)
                    mm(kp, negmu_row[0:1, tsl], csk_sb[:, k, :],
                       start=False, stop=True)
                    nc.scalar.activation(out=ks[:, 0, it, :], in_=kp,
                                         func=mybir.ActivationFunctionType.Copy,
                                         bias=0.0, scale=rstds[it][:, 0:1])
                if k + 2 < K_OPS:
                    load_wk(k + 2)

                # shifted keys in 3 DMAs (row l holds key l+1; zero padding
                # beyond the prefix makes the tail terms vanish)
                nc.sync.dma_start(out=ks[0:127, 1, :, :], in_=ks[1:128, 0, :, :])
                if pt > 1:
                    nc.sync.dma_start(out=ks[127:128, 1, 0:pt - 1, :],
                                      in_=ks[0:1, 0, 1:pt, :])
                nc.sync.dma_start(out=ks[127:128, 1, pt - 1, :], in_=zrow)

                # [G | M^T] per chunk: lhsT = keys chunk, rhs = (keys, shifted)
                # 256-wide windows; C_v^T: same lhsT, rhs = vals window.
                gm = [gm_ps.tile([128, 2, 256], f32, tag=f"gm{c}",
                                 name=f"gm{k}_{c}") for c in range(3)]
                cvt = [gm_ps.tile([128, 256], f32, tag=f"cv{c}",
                                  name=f"cvt{k}_{c}") for c in range(3)]
                for it in range(pt):
                    for c in range(3):
                        lhs = ks[:, 0, it, c * 128:(c + 1) * 128]
                        mm(gm[c], lhs, ks[:, :, it, GB[c]:GB[c] + 256],
                           start=(it == 0), stop=(it == pt - 1))
                        mm(cvt[c], lhs, vals[it][:, CVB[c]:CVB[c] + 256],
                           start=(it == 0), stop=(it == pt - 1))
                for c in range(3):
                    gm_sb = gmsb.tile([128, 2, 256], f32, tag=f"gs{c}",
                                      name=f"gmsb{k}_{c}")
                    nc.vector.tensor_copy(out=gm_sb, in_=gm[c])
                    nc.sync.dma_start(out=gm_out[k, c], in_=gm_sb)
                    cv_sb = gmsb.tile([128, 256], f32, tag=f"cs{c}",
                                      name=f"cvsb{k}_{c}")
                    nc.vector.tensor_copy(out=cv_sb, in_=cvt[c])
                    nc.sync.dma_start(out=cvt_out[k, c], in_=cv_sb)
    return _split_multi_waits(nc)


def _build_phase2():
    import concourse.bass as bass
    import concourse.tile as tile
    from concourse import mybir
    from concourse.masks import make_identity
    from contextlib import ExitStack

    f32 = mybir.dt.float32
    bf16 = mybir.dt.bfloat16

    nc = bass.Bass()
    xh = nc.dram_tensor("xh", [TH, D], bf16, kind="ExternalInput")
    weff = nc.dram_tensor("weff", [D, H * HD], bf16, kind="ExternalInput")
    wo = nc.dram_tensor("wo", [H * HD, D], bf16, kind="ExternalInput")
    cswe = nc.dram_tensor("cswe", [1, H * HD], bf16, kind="ExternalInput")
    y_out = nc.dram_tensor("y_out", [TH, D], f32, kind="ExternalOutput")

    mm = nc.tensor.matmul
    NHD = (H * HD) // 128  # 8 head-dim chunks

    with tile.TileContext(nc) as tc, ExitStack() as ctx:
        const = ctx.enter_context(tc.tile_pool(name="const", bufs=1))
        wp = ctx.enter_context(tc.tile_pool(name="wp", bufs=1))
        xtp = ctx.enter_context(tc.tile_pool(name="xtp", bufs=1))
        xin = ctx.enter_context(tc.tile_pool(name="xin", bufs=1))
        lnp = ctx.enter_context(tc.tile_pool(name="lnp", bufs=3))
        rsp = ctx.enter_context(tc.tile_pool(name="rsp", bufs=1))
        ysb = ctx.enter_context(tc.tile_pool(name="ysb", bufs=3))

        ident_f = const.tile([128, 128], f32)
        make_identity(nc, ident_f)
        ident = const.tile([128, 128], bf16)
        nc.vector.tensor_copy(out=ident, in_=ident_f)
        eps_t = const.tile([128, 1], f32)
        nc.vector.memset(eps_t, LN_EPS)
        negmu_row = const.tile([1, TH], bf16)
        rstds = [rsp.tile([128, 1], f32, tag=f"r{it}", name=f"rstd{it}")
                 for it in range(NTH)]

        xT = [xtp.tile([128, TH], bf16, tag=f"xT{d}", name=f"xT{d}")
              for d in range(ND)]
        accT = [xtp.tile([128, TH], bf16, tag=f"aT{j}", name=f"accT{j}")
                for j in range(NHD)]
        weff_sb = [wp.tile([128, H * HD], bf16, tag=f"we{i}", name=f"we{i}")
                   for i in range(ND)]
        wo_sb = wp.tile([128, NHD, D], bf16)
        cswe_sb = const.tile([1, H * HD], bf16)

        xh_r = xh.rearrange("(t p) (a b) -> t p a b", p=128, a=2)

        # DMA order: x tiles gate the transposes from t=0; weff chunks gate
        # accT; wo (by column halves) gates the y stage
        x_tiles = []
        for it in range(NTH):
            x_tile = xin.tile([128, 2, 512], bf16, tag=f"x{it}", name=f"xt{it}")
            nc.sync.dma_start(out=x_tile, in_=xh_r[it])
            x_tiles.append(x_tile)
        nc.sync.dma_start(out=cswe_sb, in_=cswe[:, :])
        for i in range(ND):
            nc.sync.dma_start(out=weff_sb[i],
                              in_=weff[i * 128:(i + 1) * 128, :])
        wo_r = wo.rearrange("(a p) n -> p a n", p=128)
        nc.sync.dma_start(out=wo_sb[:, :, 0:512], in_=wo_r[:, :, 0:512])
        nc.sync.dma_start(out=wo_sb[:, :, 512:1024], in_=wo_r[:, :, 512:1024])

        with tc.tile_pool(name="tp_ps", bufs=4, space="PSUM") as tp_ps, \
             tc.tile_pool(name="scr_ps", bufs=1, space="PSUM") as scr_ps:
            scr = scr_ps.tile([1, 1], f32)
            nc.tensor.matmul(scr, ident_f[:, 0:1], ident_f[:, 0:1],
                             start=True, stop=True)
            for it in range(NTH):
                _emit_ln_stats(nc, mybir, lnp, eps_t, x_tiles[it], it,
                               negmu_row, rstds, 1.0 / D)
                nc.tensor.matmul(scr, ident_f[:, 0:1], ident_f[:, 0:1],
                                 start=True, stop=True)
                for d in range(ND):
                    tp = tp_ps.tile([128, 128], bf16)
                    sl = x_tiles[it][:, d // 4, (d % 4) * 128:(d % 4) * 128 + 128]
                    nc.tensor.transpose(tp, sl, ident)
                    nc.vector.tensor_copy(out=xT[d][:, it * 128:(it + 1) * 128],
                                          in_=tp)

        # acc^T[j] = sum_i W_eff[i, jsl]^T x^T[i] + colsum(W_eff)_j (-mu)^T
        # i-outer (streams with the weff arrivals, 4 PSUM banks per half);
        # then y[tile] = rstd * sum_j acc^T[j, tile]^T @ W_O[j-chunk, :]
        with tc.tile_pool(name="acc_ps", bufs=1, space="PSUM") as acc_ps, \
             tc.tile_pool(name="y_ps", bufs=3, space="PSUM") as y_ps:
            for tch in range(TH // 512):
                tsl = slice(tch * 512, (tch + 1) * 512)
                for jh in range(2):
                    pss = [acc_ps.tile([128, 512], f32, tag=f"a{jj}",
                                       name=f"acc{tch}_{jh}_{jj}")
                           for jj in range(4)]
                    for i in range(ND):
                        for jj in range(4):
                            j = jh * 4 + jj
                            mm(pss[jj], weff_sb[i][:, j * 128:(j + 1) * 128],
                               xT[i][:, tsl],
                               start=(i == 0), stop=False)
                    for jj in range(4):
                        j = jh * 4 + jj
                        mm(pss[jj], cswe_sb[0:1, j * 128:(j + 1) * 128],
                           negmu_row[0:1, tsl], start=False, stop=True)
                        nc.vector.tensor_copy(
                            out=accT[j][:, tsl], in_=pss[jj])
            for it in range(NTH):
                ysl = slice(it * 128, (it + 1) * 128)
                y_sb = ysb.tile([128, 2, 512], f32)
                for ch in range(D // 512):
                    csl = slice(ch * 512, (ch + 1) * 512)
                    ps = y_ps.tile([128, 512], f32)
                    for j in range(NHD):
                        mm(ps, accT[j][:, ysl], wo_sb[:, j, csl],
                           start=(j == 0), stop=(j == NHD - 1))
                    nc.scalar.activation(out=y_sb[:, ch, :], in_=ps,
                                         func=mybir.ActivationFunctionType.Copy,
                                         bias=0.0, scale=rstds[it][:, 0:1])
                nc.sync.dma_start(
                    out=y_out[ysl, :].rearrange("p (a b) -> p a b", a=2),
                    in_=y_sb)
    return _split_multi_waits(nc)


def _numpy_fallback(hidden_states, W_K_ops, W_Q_ops, W_V, W_O, ln_gamma, ln_beta,
                    gate_alphas, gate_alpha, log_ridges, log_gammas, pl):
    x = np.asarray(hidden_states, np.float64)
    mu = x.mean(-1, keepdims=True)
    var = x.var(-1, keepdims=True)
    normed = (x - mu) / np.sqrt(var + LN_EPS) * ln_gamma + ln_beta
    values = (normed @ W_V).reshape(B, T, H, HD).transpose(0, 2, 1, 3)
    acc = np.zeros((B, H, T, HD))
    eye = np.eye(R)
    for k in range(K_OPS):
        ridge = math.exp(float(log_ridges[k]))
        gamma = math.exp(float(log_gammas[k]))
        gate = 1.0 / (1.0 + math.exp(-float(gate_alphas[k])))
        keys = (normed @ W_K_ops[k]).reshape(B, T, H, R).transpose(0, 2, 1, 3)
        qs = (normed @ W_Q_ops[k]).reshape(B, T, H, R).transpose(0, 2, 1, 3)
        pk = keys[:, :, :pl, :]
        G = np.einsum('bhlr,bhls->bhrs', pk, pk) + ridge * eye
        M = np.einsum('bhlr,bhls->bhrs', pk[:, :, 1:, :], pk[:, :, :-1, :])
        L = np.linalg.cholesky(G)
        Linv = np.linalg.inv(L)
        A = Linv @ M @ np.swapaxes(Linv, -1, -2)
        sig = np.linalg.svd(A, compute_uv=False)[..., 0]
        sig = np.maximum(sig, 1e-8)
        scale = min(gamma, 1.0) / np.maximum(sig, 1.0)
        A = A * scale[..., None, None]
        pv = values[:, :, :pl, :]
        Cv = np.einsum('bhld,bhlr->bhdr', pv, pk)
        Ginv = np.swapaxes(Linv, -1, -2) @ Linv
        Bv = Cv @ Ginv
        E = Bv @ L @ A @ A @ Linv
        out_k = np.einsum('bhdr,bhtr->bhtd', E, qs)
        acc = acc + gate * out_k
    out = acc.transpose(0, 2, 1, 3).reshape(B, T, H * HD) @ W_O
    sg = 1.0 / (1.0 + math.exp(-float(np.asarray(gate_alpha).ravel()[0])))
    return (sg * out).astype(np.float32)


def _extract_gmcv(res):
    """host-side: pull the per-head diagonal blocks out of the chunked
    [G | M^T] and C_v^T device outputs."""
    gm = res["gm_out"]      # [K, 3, 128, 2, 256]
    cvt = res["cvt_out"]    # [K, 3, 128, 256]
    G = np.empty((K_OPS, HPC, R, R), np.float64)
    M = np.empty((K_OPS, HPC, R, R), np.float64)
    Cv = np.empty((K_OPS, HPC, HD, R), np.float64)
    for h in range(HPC):
        for (c, off, nr, dst) in _blocks_for_head(h):
            gcs = slice(R * h - GB[c], R * h - GB[c] + R)
            G[:, h, dst:dst + nr, :] = gm[:, c, off:off + nr, 0, gcs]
            M[:, h, :, dst:dst + nr] = np.swapaxes(
                gm[:, c, off:off + nr, 1, gcs], -1, -2)
            vcs = slice(HD * h - CVB[c], HD * h - CVB[c] + HD)
            Cv[:, h, :, dst:dst + nr] = np.swapaxes(
                cvt[:, c, off:off + nr, vcs], -1, -2)
    return G, M, Cv


def kernel(hidden_states, W_K_ops, W_Q_ops, W_V, W_O, ln_gamma, ln_beta,
           gate_alphas, gate_alpha, log_ridges, log_gammas, prefix_len):
    from concourse.bass_utils import run_bass_kernel_spmd

    hidden_states = np.ascontiguousarray(np.asarray(hidden_states, np.float32))
    W_K_ops = np.asarray(W_K_ops, np.float32)
    W_Q_ops = np.asarray(W_Q_ops, np.float32)
    W_V = np.asarray(W_V, np.float32)
    W_O = np.ascontiguousarray(np.asarray(W_O, np.float32))
    ln_gamma = np.asarray(ln_gamma, np.float32)
    ln_beta = np.asarray(ln_beta, np.float32)
    gate_alphas = np.asarray(gate_alphas, np.float32)
    log_ridges = np.asarray(log_ridges, np.float32)
    log_gammas = np.asarray(log_gammas, np.float32)
    pl = max(1, min(int(prefix_len), T - 1))
    pt = (pl + 127) // 128

    if np.any(ln_beta != 0) or pl < 2:
        return _numpy_fallback(hidden_states, W_K_ops, W_Q_ops, W_V, W_O,
                               ln_gamma, ln_beta, gate_alphas, gate_alpha,
                               log_ridges, log_gammas, pl)

    # fold LN gamma into the projection weights; bf16 device copies
    wk_f = W_K_ops * ln_gamma[None, :, None]
    wq_f = W_Q_ops * ln_gamma[None, :, None]
    wv_f = W_V * ln_gamma[:, None]

    if pl == pt * 128:
        xpad = hidden_states[:, :pl]
    else:
        xpad = np.zeros((B, pt * 128, D), np.float32)
        xpad[:, :pl] = hidden_states[:, :pl]
    xpad_b = _bf16(xpad)

    in1 = []
    for c in range(NCORES):
        b, h0 = c // 2, (c % 2) * HPC
        wk_c = _bf16(wk_f[:, :, h0 * R:(h0 + HPC) * R])
        wv_c = _bf16(wv_f[:, h0 * HD:(h0 + HPC) * HD])
        in1.append({
            "xp": xpad_b[b],
            "wk": np.ascontiguousarray(wk_c),
            "wv": np.ascontiguousarray(wv_c),
            "csk": np.ascontiguousarray(
                wk_c.astype(np.float32).sum(1)[:, None, :]).astype(wk_c.dtype),
            "csv": np.ascontiguousarray(
                wv_c.astype(np.float32).sum(0)[None, :]).astype(wv_c.dtype),
        })

    key1 = ("p1", pt)
    if key1 not in _cache:
        _cache[key1] = _build_phase1(pt)
    r1 = run_bass_kernel_spmd(_cache[key1], in1, core_ids=list(range(NCORES)))
    LAST_PERF["p1"] = r1

    # ---- host linear algebra on 48x48 blocks -> W_eff per batch ----
    ridge = np.exp(log_ridges.astype(np.float64))
    gamma_k = np.exp(log_gammas.astype(np.float64))
    gates = 1.0 / (1.0 + np.exp(-gate_alphas.astype(np.float64)))
    sg = 1.0 / (1.0 + math.exp(-float(np.asarray(gate_alpha).ravel()[0])))
    eye = np.eye(R)

    E_full = np.empty((B, K_OPS, H, HD, R))
    for c in range(NCORES):
        b, h0 = c // 2, (c % 2) * HPC
        G, M, Cv = _extract_gmcv(r1.results[c])
        G = G + ridge[:, None, None, None] * eye
        L = np.linalg.cholesky(G)
        Linv = np.linalg.inv(L)
        A = Linv @ M @ np.swapaxes(Linv, -1, -2)
        sig = np.linalg.svd(A, compute_uv=False)[..., 0]
        sig = np.maximum(sig, 1e-8)
        scale = np.minimum(gamma_k, 1.0)[:, None] / np.maximum(sig, 1.0)
        A = A * scale[..., None, None]
        Ginv = np.swapaxes(Linv, -1, -2) @ Linv
        Bv = Cv @ Ginv
        E = Bv @ L @ A @ A @ Linv          # [K, HPC, HD, R]
        E = E * (sg * gates)[:, None, None, None]
        E_full[b, :, h0:h0 + HPC] = E

    # W_eff[b] = sum_k Wq_f[k][:, h-block] @ E[b, k, h]^T   -> [D, H*HD]
    wq_h = wq_f.reshape(K_OPS, D, H, R).transpose(0, 2, 1, 3)  # [K, H, D, R]
    weffs = []
    for b in range(B):
        w = np.zeros((H, D, HD), np.float64)
        for k in range(K_OPS):
            w += wq_h[k].astype(np.float64) @ E_full[b, k].transpose(0, 2, 1)
        weffs.append(np.ascontiguousarray(
            w.transpose(1, 0, 2).reshape(D, H * HD)))

    xh_b = _bf16(hidden_states)
    wo_b = _bf16(W_O)
    in2 = []
    for c in range(NCORES):
        b, th = c // 2, c % 2
        weff_b = _bf16(weffs[b])
        in2.append({
            "xh": np.ascontiguousarray(xh_b[b, th * TH:(th + 1) * TH]),
            "weff": weff_b,
            "wo": wo_b,
            "cswe": np.ascontiguousarray(
                weffs[b].sum(0)[None, :].astype(np.float32)).astype(weff_b.dtype),
        })

    if "p2" not in _cache:
        _cache["p2"] = _build_phase2()
    r2 = run_bass_kernel_spmd(_cache["p2"], in2, core_ids=list(range(NCORES)))
    LAST_PERF["p2"] = r2

    y = np.empty((B, T, D), np.float32)
    for c in range(NCORES):
        b, th = c // 2, c % 2
        y[b, th * TH:(th + 1) * TH] = r2.results[c]["y_out"]
    return y
